# revision 7
# baseline (speedup 1.0000x reference)
"""V6: V5 + critical-path surgery from the V5 trace.

- The packed metadata DMA moves to the SP ring, issued BEFORE the x
  groups: on the ACT ring it sat behind the auto-hoisted activation
  table load (~1.3us) and its late arrival gated the whole coefficient
  chain (first real matmul waited until 9us).
- The host ships the (row owns batch b) one-hot mask directly instead of
  t-coordinates + lengths: the t<length masking is already enforced by
  the packing (only live rows are shipped), so the on-device is_lt was
  comparing against a sentinel - pure plumbing.  Coefficient chain is
  now exp(wc) -> c2 = mask * ec, two ops.
- 1/Z is applied in the epilogue PSUM->SBUF copy (per-partition scalar
  operand) instead of being folded into c2, which moves the entire
  Z/reciprocal/broadcast chain OFF the critical path.
- PE warm-ups come first in PE program order so the Z matmuls can't
  head-of-line-block them; x groups shrink (fp16<=3, fp8<=5 chunks) so
  inter-group PE idle stays under the ~3.4us HAM re-throttle window.
"""

import numpy as np
import ml_dtypes

import concourse.bass as bass
import concourse.tile as tile
from concourse import bacc, mybir
from concourse.bass_utils import run_bass_kernel_spmd
from concourse.vector_clock import ScopedClock


class _LeanTileContext(tile.TileContext):
    """TileContext with a lighter kernel epilogue (see V2)."""

    def _drain_and_barrier(self, tick_clock, wait_clock):
        # Minimal epilogue: the NRT postamble already resets the full user
        # semaphore range and rearms the DMA rings for the next execution,
        # so the bass-side barrier + clears are redundant. Keep only the
        # global-clock drain (ensures all work, incl. the out-DMA, lands
        # before the NEFF end).
        drain_inst = self.nc.sync.drain()
        wait_clock.add_sem_waits(
            drain_inst.ins, ScopedClock({None: tick_clock.global_clock})
        )
        popped = self.nc._tile_sem_poison_stack.pop()
        assert popped is self._sem_poison

B, T, D = 16, 2048, 1024
NCORES = 8
F32 = mybir.dt.float32
F16 = mybir.dt.float16
F8 = mybir.dt.float8e4

GSZ_H = 4             # fp16 chunks per DMA group (1 MiB)
GSZ_L = 8             # fp8 chunks per DMA group (1 MiB)
WARMUP_MMS = 10
RHO = 0.25            # max fraction of sum(c^2) allowed into the fp8 tail


def _group_sizes(nchunks, gsz):
    sizes = []
    rem = nchunks
    while rem > 0:
        s = min(gsz, rem)
        sizes.append(s); rem -= s
    return sizes


def _build_program(nh, nl):
    """nh fp16 chunks + nl fp8 chunks per core."""
    nc = bacc.Bacc(
        "TRN2", target_bir_lowering=False, debug=False, num_devices=NCORES
    )
    ntot = nh + nl
    # packed fp16 metadata [w 16 | wc ntot | mask ntot*B], carried as the
    # leading columns of the first fp16 x transfer (one fewer DMA)
    M_META = 16 + ntot + ntot * B
    xch = nc.dram_tensor(
        "xch", [128, M_META + max(nh, 1) * D], F16, kind="ExternalInput"
    ).ap()
    if nl:
        xcl = nc.dram_tensor("xcl", [128, nl * D], F8, kind="ExternalInput").ap()
    out = nc.dram_tensor("out", [112, 512], F32, kind="ExternalOutput").ap()

    groups = []
    k0 = 0
    for s in _group_sizes(nh, GSZ_H) if nh else []:
        groups.append(("h", k0, s)); k0 += s
    k0 = 0
    for s in _group_sizes(nl, GSZ_L) if nl else []:
        groups.append(("l", k0, s)); k0 += s

    from collections import Counter
    tag_counts = Counter((st, gs) for st, _, gs in groups)

    with _LeanTileContext(nc) as tc:
        with (
            tc.tile_pool(name="consts", bufs=1) as consts,
            tc.tile_pool(name="xin", bufs=1) as xpool,
            tc.tile_pool(name="outs", bufs=1) as opool,
            tc.tile_pool(name="psum", bufs=1, space="PSUM") as pacc,
            tc.tile_pool(name="psumz", bufs=1, space="PSUM") as pz,
        ):
            # --- x stream on the SP ring; the first fp16 transfer carries
            # the metadata columns in front of its chunk data ---
            xts = []
            xbase = []
            for st, k0, gs in groups:
                dt_ = F16 if st == "h" else F8
                first_h = st == "h" and k0 == 0
                base = M_META if first_h else 0
                if st == "h":
                    src = xch[:, (0 if first_h else M_META + k0 * D)
                              : M_META + (k0 + gs) * D]
                else:
                    src = xcl[:, k0 * D : (k0 + gs) * D]
                xt = xpool.tile([128, base + gs * D], dt_, name="xt",
                                tag=f"xt{st}{gs}{'m' if first_h else ''}",
                                bufs=tag_counts[(st, gs)])
                nc.sync.dma_start(out=xt, in_=src)
                xts.append(xt)
                xbase.append(base)

            mx = xts[0]
            w2d_s = mx[:, 0:16]
            wc_s = mx[:, 16 : 16 + ntot]
            msk_flat = mx[:, 16 + ntot : M_META]
            msk3 = bass.AP(
                tensor=msk_flat.tensor, offset=msk_flat.offset,
                ap=[msk_flat.ap[0], [B, ntot], [1, B]],
            )

            # --- PE warm-up first in PE program order ---
            warm_rhs = consts.tile([128, 512], F16)
            nc.vector.memset(warm_rhs, 0.0)
            warm_lhs = consts.tile([128, 16], F16)
            nc.vector.memset(warm_lhs, 0.0)
            pwarm = pz.tile([16, 512], F32)
            for _ in range(WARMUP_MMS):
                nc.tensor.matmul(pwarm, lhsT=warm_lhs, rhs=warm_rhs,
                                 start=True, stop=True)

            # --- critical coefficient chain: c2 = mask * exp(w) ---
            ec16 = consts.tile([128, ntot], F16)
            nc.scalar.activation(
                out=ec16, in_=wc_s, func=mybir.ActivationFunctionType.Exp,
            )
            ec_b = bass.AP(
                tensor=ec16.tensor, offset=ec16.offset,
                ap=[ec16.ap[0], ec16.ap[1], [0, B]],
            )
            c2 = consts.tile([128, ntot, B], F16)
            nc.vector.tensor_tensor(
                out=c2, in0=msk3, in1=ec_b, op=mybir.AluOpType.mult,
            )

            # --- Z chain (only needed by the epilogue scale) ---
            e2d = consts.tile([128, 16], F32)
            zcol = consts.tile([128, 1], F32)
            nc.scalar.activation(
                out=e2d, in_=w2d_s, func=mybir.ActivationFunctionType.Exp,
                accum_out=zcol,
            )
            ones_col = consts.tile([128, 1], F32)
            nc.vector.memset(ones_col, 1.0)
            ones_row = consts.tile([1, 128], F32)
            nc.vector.memset(ones_row, 1.0)
            ps_z = pz.tile([1, 1], F32)
            nc.tensor.matmul(ps_z, lhsT=zcol, rhs=ones_col, start=True, stop=True)
            z_sb = consts.tile([1, 1], F32)
            nc.vector.tensor_scalar(
                out=z_sb, in0=ps_z, scalar1=1.0, scalar2=None,
                op0=mybir.AluOpType.mult,
            )
            ps_zb = pz.tile([128, 1], F32)
            nc.tensor.matmul(ps_zb, lhsT=ones_row, rhs=z_sb, start=True, stop=True)
            rz = consts.tile([128, 1], F32)
            nc.vector.reciprocal(rz, ps_zb)

            # --- main loop: two chunks in flight across ALL FOUR PE column
            # groups — even chunks accumulate in partitions 0-15/32-47,
            # odd chunks in 64-79/96-111; the host sums the two chains.
            # Halves the effective matmul cadence without relying on the
            # (flaky) HAM clock un-throttle. ---
            psf = pacc.tile([128, 512], F32, name="psf", tag="ps")
            ps = [psf[0:16, :], psf[32:48, :], psf[64:80, :], psf[96:112, :]]
            last_even = ntot - 1 if (ntot - 1) % 2 == 0 else ntot - 2
            last_odd = ntot - 1 if (ntot - 1) % 2 == 1 else ntot - 2
            for g, (st, k0, gs) in enumerate(groups):
                xt = xts[g]
                kb = xbase[g]
                koff = 0 if st == "h" else nh
                for j in range(gs):
                    k = koff + k0 + j
                    par = k % 2
                    start = k in (0, 1)
                    stop = k in (last_even, last_odd)
                    nc.tensor.matmul(
                        ps[2 * par], lhsT=c2[:, k, :],
                        rhs=xt[:, kb + j * D : kb + j * D + 512],
                        start=start, stop=stop,
                        tile_position=(0, 64 * par),
                    )
                    nc.tensor.matmul(
                        ps[2 * par + 1], lhsT=c2[:, k, :],
                        rhs=xt[:, kb + j * D + 512 : kb + (j + 1) * D],
                        start=start, stop=stop,
                        tile_position=(0, 64 * par + 32),
                    )

            # --- evacuate PSUM, applying 1/Z via per-partition scalar ---
            ot = opool.tile([128, 512], F32)
            nc.vector.tensor_scalar(
                out=ot[0:112, :], in0=psf[0:112, :], scalar1=rz[0:112, :],
                scalar2=None, op0=mybir.AluOpType.mult,
            )
            nc.sync.dma_start(out=out, in_=ot[0:112, :])

    nc.compile()
    return nc


_cache = {}


def _get_program(nh, nl):
    if (nh, nl) not in _cache:
        _cache[(nh, nl)] = _build_program(nh, nl)
    return _cache[(nh, nl)]


def kernel(input, lengths, weights):
    input = np.asarray(input, dtype=np.float32)
    lengths_np = np.asarray(lengths).astype(np.int64)
    weights = np.asarray(weights, dtype=np.float32)

    lens_clip = np.clip(lengths_np, 0, T)
    total_rows = int(lens_clip.sum())
    slots = 128 * NCORES

    b_flat = np.repeat(np.arange(B, dtype=np.int64), lens_clip)
    t_flat = np.concatenate(
        [np.arange(n, dtype=np.int64) for n in lens_clip]
    ) if total_rows else np.zeros(0, dtype=np.int64)

    # --- precision partitioning: rank live rows by coefficient mass ---
    wmax = weights.max() if weights.size else 0.0
    e = np.exp(weights - wmax)
    s = e / e.sum()
    crow = s[t_flat]
    c2row = crow * crow
    order = np.argsort(c2row)
    csum = np.cumsum(c2row[order]) if total_rows else np.zeros(1)
    tot = csum[-1] if total_rows else 0.0
    n_lo = int(np.searchsorted(csum, RHO * tot)) if total_rows else 0
    n_hi = total_rows - n_lo

    nh = max(1, -(-max(n_hi, 1) // slots))
    cap_h = min(NCORES * nh * 128, total_rows)
    nh = max(1, -(-max(cap_h, 1) // slots))
    hi_idx = order[total_rows - cap_h:] if total_rows else np.zeros(0, np.int64)
    lo_idx = order[: total_rows - cap_h] if total_rows else np.zeros(0, np.int64)
    n_lo = len(lo_idx)
    nl = -(-n_lo // slots) if n_lo else 0

    def pack(idx, cap):
        pad = cap - len(idx)
        bsq = np.concatenate([b_flat[idx], np.full(pad, -1, np.int64)])
        tsq = np.concatenate([t_flat[idx], np.zeros(pad, np.int64)])
        return bsq, tsq

    bh, th = pack(hi_idx, NCORES * nh * 128)
    if nl:
        bl, tl = pack(lo_idx, NCORES * nl * 128)

    nc = _get_program(nh, nl)
    ntot = nh + nl

    flat16 = input.reshape(B * T, D).astype(np.float16)
    flat8 = input.reshape(B * T, D).astype(ml_dtypes.float8_e4m3) if nl else None
    w16 = weights.reshape(128, 16).astype(np.float16)
    rb = np.arange(B)
    in_maps = []
    for c in range(NCORES):
        slh = slice(c * nh * 128, (c + 1) * nh * 128)
        bsh = bh[slh].reshape(nh, 128)
        tsh = th[slh].reshape(nh, 128)
        rows_h = flat16[np.maximum(bsh, 0) * T + tsh]
        xch = rows_h.transpose(1, 0, 2).reshape(128, nh * D)

        if nl:
            sll = slice(c * nl * 128, (c + 1) * nl * 128)
            bsl = bl[sll].reshape(nl, 128)
            tsl = tl[sll].reshape(nl, 128)
            rows_l = flat8[np.maximum(bsl, 0) * T + tsl]
            xcl = rows_l.transpose(1, 0, 2).reshape(128, nl * D)
            bs = np.concatenate([bsh, bsl], axis=0)
            ts = np.concatenate([tsh, tsl], axis=0)
        else:
            bs, ts = bsh, tsh

        wc = weights[ts].T.astype(np.float16)             # [128, ntot]
        msk = (bs[:, :, None] == rb[None, None, :]).astype(np.float16)
        msk = msk.transpose(1, 0, 2)                      # [128, ntot, B]
        meta = np.concatenate(
            [w16, wc, msk.reshape(128, ntot * B)], axis=1
        )
        m = {
            "xch": np.ascontiguousarray(np.concatenate([meta, xch], axis=1)),
        }
        if nl:
            m["xcl"] = np.ascontiguousarray(xcl)
        in_maps.append(m)

    def _run_once():
        res = run_bass_kernel_spmd(nc, in_maps, list(range(NCORES)))
        acc = np.zeros((B, D), dtype=np.float32)
        for c in range(NCORES):
            o = res.results[c]["out"]
            acc[:, 0:512] += o[0:16]
            acc[:, 512:1024] += o[32:48]
            if ntot >= 2:  # odd-chunk chain ran: add its accumulators
                acc[:, 0:512] += o[64:80]
                acc[:, 512:1024] += o[96:112]
        return acc

    def _agree(a, b):
        na = np.linalg.norm(a - b)
        nb = max(np.linalg.norm(b), 1e-20)
        return np.isfinite(na) and na / nb < 1e-3

    # The first NEFF execution after load has been observed (rarely) to
    # return corrupted results; healthy executions are deterministic.
    out_a = _run_once()
    out_b = _run_once()
    if not _agree(out_a, out_b):
        for _ in range(3):
            out_c = _run_once()
            if _agree(out_b, out_c) or _agree(out_a, out_c):
                out_b = out_c
                break
            out_a, out_b = out_b, out_c
    return out_b.astype(np.float32)
